# revision 12
# baseline (speedup 1.0000x reference)
"""BertSelfAttention (B=4, S=2048, D=1024, H=16) on 8 Trainium2 NeuronCores.

Sharding (no collectives needed):
  core c -> batch b = c // 2, head-group g = c % 2 (heads g*8 .. g*8+8,
  i.e. columns g*512 .. (g+1)*512 of the QKV projections and of the output).
  Each core computes the full attention for its 8 heads of its batch and
  writes a [2048, 512] slice of the output; the host reassembles.

Per-core kernel (all matmuls bf16 with fp32 PSUM accumulation):
  1. x [2048,1024] f32 -> cast-DMA -> xb16 (HBM scratch) -> DMA-transpose
     -> xT in SBUF as 8 x [128, 2048] (contraction dim on partitions).
  2. qT = (Wq^T x^T) [512, 2048], kT likewise, v = (x Wv) [2048, 512+ones]
     computed with TensorE, evicted to bf16 SBUF via VectorE (+bias).
  3. Per head h, q-block (1024 wide): scoresT tile sT[k,q] = kT_h^T qT_h
     (K=64 contraction; two heads packed on PE row-groups 0-1/2-3),
     exp via ScalarE: e = Exp(0.125*sT + mask[k]) -> bf16,
     out_aug[q, 0:65] += e_slice^T @ [v_h | 1] accumulated over k-blocks
     (column 64 accumulates the softmax denominator),
  4. out = out_aug[:, 0:64] * (1/out_aug[:, 64]) + bv  (VectorE), DMA out.

softmax max-subtraction is skipped deliberately: scores = (q.k)/8 with the
problem's fixed input distribution stay in [-6, 6], so exp() is safe in
fp32/bf16 range.
"""

import numpy as np

import concourse.bass as bass
import concourse.mybir as mybir
import concourse.tile as tile
from concourse import bacc
from concourse.bass_utils import run_bass_kernel_spmd
from concourse.masks import make_identity

B, S, D, H = 4, 2048, 1024, 16
HD = D // H            # 64
NCORES = 8
DC = 512               # projection columns handled per core
HC = 8                 # heads per core
VW = HD + 1            # v columns per head incl. the ones column (65)

f32 = mybir.dt.float32
bf16 = mybir.dt.bfloat16

_cache: dict = {}


def _build() -> bass.Bass:
    AF = mybir.ActivationFunctionType
    nc = bacc.Bacc("TRN2", target_bir_lowering=False, debug=False)

    x_d = nc.dram_tensor("x", [S, D], f32, kind="ExternalInput").ap()
    wq_d = nc.dram_tensor("wq", [D, DC], f32, kind="ExternalInput").ap()
    wk_d = nc.dram_tensor("wk", [D, DC], f32, kind="ExternalInput").ap()
    wv_d = nc.dram_tensor("wv", [D, DC], f32, kind="ExternalInput").ap()
    bq_d = nc.dram_tensor("bq", [DC], f32, kind="ExternalInput").ap()
    bk_d = nc.dram_tensor("bk", [DC], f32, kind="ExternalInput").ap()
    bv_d = nc.dram_tensor("bv", [DC], f32, kind="ExternalInput").ap()
    mask_d = nc.dram_tensor("mask", [S], f32, kind="ExternalInput").ap()
    out_d = nc.dram_tensor("out", [S, DC], f32, kind="ExternalOutput").ap()

    with tile.TileContext(nc) as tc:
        _emit(nc, tc, x_d, wq_d, wk_d, wv_d, bq_d, bk_d, bv_d, mask_d, out_d, AF)
    nc.compile()
    return nc


def _emit(nc, tc, x_d, wq_d, wk_d, wv_d, bq_d, bk_d, bv_d, mask_d, out_d, AF):
    from contextlib import ExitStack

    with ExitStack() as ctx:
        dram = ctx.enter_context(tc.tile_pool(name="dram", bufs=1, space="DRAM"))
        const = ctx.enter_context(tc.tile_pool(name="const", bufs=1))
        persist = ctx.enter_context(tc.tile_pool(name="persist", bufs=1))

        # ---------------- constants ----------------
        mask_sb = const.tile([128, S // 128], f32, name="mask_sb")
        nc.sync.dma_start(out=mask_sb[:], in_=mask_d.rearrange("(n p) -> p n", p=128))
        bq_sb = const.tile([128, DC // 128], f32, name="bq_sb")
        nc.sync.dma_start(out=bq_sb[:], in_=bq_d.rearrange("(n p) -> p n", p=128))
        bk_sb = const.tile([128, DC // 128], f32, name="bk_sb")
        nc.sync.dma_start(out=bk_sb[:], in_=bk_d.rearrange("(n p) -> p n", p=128))
        bv_row = const.tile([1, DC], f32, name="bv_row")
        nc.sync.dma_start(out=bv_row[:], in_=bv_d.rearrange("(a d) -> a d", a=1))
        ones_row = const.tile([1, 128], f32, name="ones_row")
        nc.vector.memset(ones_row[:], 1.0)
        bv_bc = const.tile([128, DC], f32, name="bv_bc")

        # persistent activation tensors
        qT = [persist.tile([128, S], bf16, name=f"qT{m}") for m in range(4)]
        kT = [persist.tile([128, S], bf16, name=f"kT{m}") for m in range(4)]
        v_sb = [persist.tile([128, HC * VW], bf16, name=f"v{m}") for m in range(16)]

        # ---------------- phase 0+1: load, transpose, project ----------------
        with (
            tc.tile_pool(name="projin", bufs=1) as projin,
            tc.tile_pool(name="pj_psum", bufs=3, space="PSUM") as pj,
        ):
            # weights cast f32 -> bf16 during DMA
            wq_sb = [projin.tile([128, DC], bf16, name=f"wq{p}") for p in range(8)]
            wk_sb = [projin.tile([128, DC], bf16, name=f"wk{p}") for p in range(8)]
            wv_sb = [projin.tile([128, DC], bf16, name=f"wv{p}") for p in range(8)]
            for p in range(8):
                nc.gpsimd.dma_start(out=wq_sb[p][:], in_=wq_d[p * 128:(p + 1) * 128, :])
                nc.gpsimd.dma_start(out=wk_sb[p][:], in_=wk_d[p * 128:(p + 1) * 128, :])
                nc.gpsimd.dma_start(out=wv_sb[p][:], in_=wv_d[p * 128:(p + 1) * 128, :])

            # x cast-loaded to bf16 in natural layout, then transposed on the
            # TensorEngine into xT (8 transposes packed per PSUM bank via the
            # lazy-zero-region: start=True only on the first, stop on the last).
            ident = const.tile([128, 128], bf16, name="ident")
            make_identity(nc, ident[:])
            xT = [projin.tile([128, S], bf16, name=f"xT{p}") for p in range(8)]
            with tc.tile_pool(name="xnat", bufs=16) as xnp, \
                 tc.tile_pool(name="tr_psum", bufs=3, space="PSUM") as trp:
                xnat = []
                for m in range(16):
                    xn = xnp.tile([128, D], bf16, name=f"xn{m}", tag="xn")
                    nc.gpsimd.dma_start(out=xn[:], in_=x_d[m * 128:(m + 1) * 128, :])
                    xnat.append(xn)
                for p in range(8):
                    for half in range(2):
                        tp = trp.tile([128, 1024], bf16, name=f"tr{p}_{half}", tag="tr")
                        for mm in range(8):
                            m = half * 8 + mm
                            nc.tensor.matmul(
                                tp[:, mm * 128:(mm + 1) * 128],
                                xnat[m][:, p * 128:(p + 1) * 128],
                                ident[:],
                                is_transpose=True,
                                start=(mm == 0),
                                stop=(mm == 7),
                            )
                        nc.vector.tensor_copy(
                            xT[p][:, half * 1024:(half + 1) * 1024], tp[:]
                        )

            # bv broadcast to all 128 partitions via a rank-1 matmul
            bc_ps = pj.tile([128, DC], f32, name="bv_ps", tag="pj")
            nc.tensor.matmul(bc_ps[:], ones_row[:], bv_row[:], start=True, stop=True)
            nc.vector.tensor_copy(bv_bc[:], bc_ps[:])

            # qT[d', s] and kT[d', s]
            for wsb, bsb, dst in ((wq_sb, bq_sb, qT), (wk_sb, bk_sb, kT)):
                for m in range(4):
                    for n in range(4):
                        ps = pj.tile([128, 512], f32, name=f"pj_{m}_{n}", tag="pj")
                        for p in range(8):
                            nc.tensor.matmul(
                                ps[:],
                                wsb[p][:, m * 128:(m + 1) * 128],
                                xT[p][:, n * 512:(n + 1) * 512],
                                start=(p == 0),
                                stop=(p == 7),
                            )
                        nc.vector.tensor_scalar_add(
                            dst[m][:, n * 512:(n + 1) * 512], ps[:], bsb[:, m:m + 1]
                        )

            # v[s, d'] with the interleaved ones column per head
            for m in range(16):
                ones_ap = v_sb[m][:].rearrange("p (h c) -> p h c", c=VW)[:, :, HD:HD + 1]
                nc.vector.memset(ones_ap, 1.0)
                ps = pj.tile([128, 512], f32, name=f"pv_{m}", tag="pj")
                for p in range(8):
                    nc.tensor.matmul(
                        ps[:],
                        xT[p][:, m * 128:(m + 1) * 128],
                        wv_sb[p][:],
                        start=(p == 0),
                        stop=(p == 7),
                    )
                nc.vector.tensor_copy(
                    v_sb[m][:].rearrange("p (h c) -> p h c", c=VW)[:, :, 0:HD],
                    ps[:].rearrange("p (h c) -> p h c", c=HD),
                )

        # ---------------- phase 2: attention ----------------
        # Per (head-pair hp, 1024-wide query block jq):
        #   A) 16 key blocks: sT = kT_h^T qT_h (heads packed on PE row
        #      groups), e[ik][h01] = Exp(0.125*sT + mask) -> bf16 SBUF.
        #   B) per 128-query sub-block jj: one PSUM bank accumulates BOTH
        #      heads' [q,65] out_aug over all 16 key blocks (single
        #      accumulation chain per bank: start once, stop once).
        #   C) normalize by the ones-column and add bv into the staging
        #      tile; one big DMA per 512 query rows at the end of jq.
        with (
            tc.tile_pool(name="qk_psum", bufs=2, space="PSUM") as qkp,
            tc.tile_pool(name="pv_psum", bufs=3, space="PSUM") as pvp,
            tc.tile_pool(name="exp_sb", bufs=36) as ep,
            tc.tile_pool(name="stage", bufs=2) as stp,
            tc.tile_pool(name="recip", bufs=8) as rp,
        ):
            for jq in range(2):
                q0 = jq * 1024
                stage = [
                    stp.tile([128, 4 * DC], f32, name=f"st{jq}_{t}", tag=f"st{t}")
                    for t in range(2)
                ]
                for hp in range(4):
                    # --- A: scoresT + exp, both heads of the pair ---
                    e_tiles = []
                    for ik in range(16):
                        qk = [
                            qkp.tile([128, 1024], f32,
                                     name=f"qk{jq}_{hp}_{ik}_{i}", tag="qk")
                            for i in range(2)
                        ]
                        for half in range(2):
                            for h01 in range(2):
                                ro = h01 * 64
                                nc.tensor.matmul(
                                    qk[h01][:, half * 512:(half + 1) * 512],
                                    kT[hp][ro:ro + 64, ik * 128:(ik + 1) * 128],
                                    qT[hp][ro:ro + 64,
                                           q0 + half * 512:q0 + (half + 1) * 512],
                                    start=True,
                                    stop=True,
                                )
                        epair = []
                        for h01 in range(2):
                            e = ep.tile([128, 1024], bf16,
                                        name=f"e{jq}_{hp}_{ik}_{h01}", tag="e")
                            nc.scalar.activation(
                                e[:], qk[h01][:], AF.Exp,
                                bias=mask_sb[:, ik:ik + 1], scale=0.125,
                            )
                            epair.append(e)
                        e_tiles.append(epair)
                    # --- B+C: accumulate PV per 128-query sub-block ---
                    for jj in range(8):
                        t, j4 = divmod(jj, 4)
                        pv = pvp.tile([128, 2 * VW], f32,
                                      name=f"pv{jq}_{hp}_{jj}", tag="pv")
                        for h01 in range(2):
                            h = hp * 2 + h01
                            for ik in range(16):
                                nc.tensor.matmul(
                                    pv[:, h01 * VW:(h01 + 1) * VW],
                                    e_tiles[ik][h01][:, jj * 128:(jj + 1) * 128],
                                    v_sb[ik][:, h * VW:(h + 1) * VW],
                                    start=(h01 == 0 and ik == 0),
                                    stop=(h01 == 1 and ik == 15),
                                )
                        pv3 = pv[:].rearrange("p (g c) -> p g c", c=VW)
                        rc_t = rp.tile([128, 2, 1], f32,
                                       name=f"rc{jq}_{hp}_{jj}", tag="rc")
                        nc.vector.reciprocal(rc_t[:], pv3[:, :, HD:HD + 1])
                        dst = stage[t][:, j4 * DC + hp * 128:
                                       j4 * DC + (hp + 1) * 128].rearrange(
                            "p (g d) -> p g d", d=HD)
                        nc.vector.tensor_mul(
                            dst, pv3[:, :, 0:HD],
                            rc_t[:].broadcast_to([128, 2, HD]),
                        )
                        nc.vector.tensor_add(
                            dst, dst,
                            bv_bc[:, hp * 128:(hp + 1) * 128].rearrange(
                                "p (g d) -> p g d", d=HD),
                        )
                for t in range(2):
                    r0 = q0 + t * 512
                    nc.sync.dma_start(
                        out=out_d[r0:r0 + 512, :].rearrange("(j p) d -> p j d", p=128),
                        in_=stage[t][:].rearrange("p (j d) -> p j d", d=DC),
                    )


def _input_maps(input_tensor, attention_mask, Wq, bq, Wk, bk, Wv, bv):
    x = np.asarray(input_tensor, dtype=np.float32)
    m = np.asarray(attention_mask, dtype=np.float32)
    Wq = np.asarray(Wq, dtype=np.float32)
    Wk = np.asarray(Wk, dtype=np.float32)
    Wv = np.asarray(Wv, dtype=np.float32)
    bq = np.asarray(bq, dtype=np.float32)
    bk = np.asarray(bk, dtype=np.float32)
    bv = np.asarray(bv, dtype=np.float32)
    maps = []
    for c in range(NCORES):
        b, g = divmod(c, 2)
        cs = slice(g * DC, (g + 1) * DC)
        maps.append({
            "x": np.ascontiguousarray(x[b]),
            "mask": np.ascontiguousarray(m[b, 0, 0]),
            "wq": np.ascontiguousarray(Wq[:, cs]),
            "wk": np.ascontiguousarray(Wk[:, cs]),
            "wv": np.ascontiguousarray(Wv[:, cs]),
            "bq": np.ascontiguousarray(bq[cs]),
            "bk": np.ascontiguousarray(bk[cs]),
            "bv": np.ascontiguousarray(bv[cs]),
        })
    return maps


def get_nc() -> bass.Bass:
    if "nc" not in _cache:
        _cache["nc"] = _build()
    return _cache["nc"]


def kernel(input_tensor, attention_mask, Wq, bq, Wk, bk, Wv, bv, _run_kwargs=None):
    nc = get_nc()
    maps = _input_maps(input_tensor, attention_mask, Wq, bq, Wk, bk, Wv, bv)
    res = run_bass_kernel_spmd(nc, maps, list(range(NCORES)), **(_run_kwargs or {}))
    out = np.empty((B, S, D), dtype=np.float32)
    for c in range(NCORES):
        b, g = divmod(c, 2)
        out[b, :, g * DC:(g + 1) * DC] = res.results[c]["out"]
    if _run_kwargs:
        _cache["last_results"] = res
    return out


# revision 15
# speedup vs baseline: 1.0379x; 1.0379x over previous
"""BertSelfAttention (B=4, S=2048, D=1024, H=16) on 8 Trainium2 NeuronCores.

Sharding (no collectives needed):
  core c -> batch b = c // 2, head-group g = c % 2 (heads g*8 .. g*8+8,
  i.e. columns g*512 .. (g+1)*512 of the QKV projections and of the output).
  Each core computes the full attention for its 8 heads of its batch and
  writes a [2048, 512] slice of the output; the host reassembles.

Per-core kernel (all matmuls bf16 with fp32 PSUM accumulation):
  1. x [2048,1024] f32 -> cast-DMA -> xb16 (HBM scratch) -> DMA-transpose
     -> xT in SBUF as 8 x [128, 2048] (contraction dim on partitions).
  2. qT = (Wq^T x^T) [512, 2048], kT likewise, v = (x Wv) [2048, 512+ones]
     computed with TensorE, evicted to bf16 SBUF via VectorE (+bias).
  3. Per head h, q-block (1024 wide): scoresT tile sT[k,q] = kT_h^T qT_h
     (K=64 contraction; two heads packed on PE row-groups 0-1/2-3),
     exp via ScalarE: e = Exp(0.125*sT + mask[k]) -> bf16,
     out_aug[q, 0:65] += e_slice^T @ [v_h | 1] accumulated over k-blocks
     (column 64 accumulates the softmax denominator),
  4. out = out_aug[:, 0:64] * (1/out_aug[:, 64]) + bv  (VectorE), DMA out.

softmax max-subtraction is skipped deliberately: scores = (q.k)/8 with the
problem's fixed input distribution stay in [-6, 6], so exp() is safe in
fp32/bf16 range.
"""

import numpy as np

import concourse.bass as bass
import concourse.mybir as mybir
import concourse.tile as tile
from concourse import bacc
from concourse.bass_utils import run_bass_kernel_spmd
from concourse.masks import make_identity

B, S, D, H = 4, 2048, 1024, 16
HD = D // H            # 64
NCORES = 8
DC = 512               # projection columns handled per core
HC = 8                 # heads per core
VW = HD + 1            # v columns per head incl. the ones column (65)

f32 = mybir.dt.float32
bf16 = mybir.dt.bfloat16

_cache: dict = {}


def _build() -> bass.Bass:
    AF = mybir.ActivationFunctionType
    nc = bacc.Bacc("TRN2", target_bir_lowering=False, debug=False)

    x_d = nc.dram_tensor("x", [S, D], f32, kind="ExternalInput").ap()
    wq_d = nc.dram_tensor("wq", [D, DC], f32, kind="ExternalInput").ap()
    wk_d = nc.dram_tensor("wk", [D, DC], f32, kind="ExternalInput").ap()
    wv_d = nc.dram_tensor("wv", [D, DC], f32, kind="ExternalInput").ap()
    bq_d = nc.dram_tensor("bq", [DC], f32, kind="ExternalInput").ap()
    bk_d = nc.dram_tensor("bk", [DC], f32, kind="ExternalInput").ap()
    bv_d = nc.dram_tensor("bv", [DC], f32, kind="ExternalInput").ap()
    mask_d = nc.dram_tensor("mask", [S], f32, kind="ExternalInput").ap()
    out_d = nc.dram_tensor("out", [S, DC], f32, kind="ExternalOutput").ap()

    with tile.TileContext(nc) as tc:
        _emit(nc, tc, x_d, wq_d, wk_d, wv_d, bq_d, bk_d, bv_d, mask_d, out_d, AF)
    nc.compile()
    return nc


def _emit(nc, tc, x_d, wq_d, wk_d, wv_d, bq_d, bk_d, bv_d, mask_d, out_d, AF):
    from contextlib import ExitStack

    with ExitStack() as ctx:
        dram = ctx.enter_context(tc.tile_pool(name="dram", bufs=1, space="DRAM"))
        const = ctx.enter_context(tc.tile_pool(name="const", bufs=1))
        persist = ctx.enter_context(tc.tile_pool(name="persist", bufs=1))

        # ---------------- constants ----------------
        mask_sb = const.tile([128, S // 128], f32, name="mask_sb")
        nc.sync.dma_start(out=mask_sb[:], in_=mask_d.rearrange("(n p) -> p n", p=128))
        bq_sb = const.tile([128, DC // 128], f32, name="bq_sb")
        nc.sync.dma_start(out=bq_sb[:], in_=bq_d.rearrange("(n p) -> p n", p=128))
        bk_sb = const.tile([128, DC // 128], f32, name="bk_sb")
        nc.sync.dma_start(out=bk_sb[:], in_=bk_d.rearrange("(n p) -> p n", p=128))
        bv_row = const.tile([1, DC], f32, name="bv_row")
        nc.sync.dma_start(out=bv_row[:], in_=bv_d.rearrange("(a d) -> a d", a=1))
        ones_row = const.tile([1, 128], f32, name="ones_row")
        nc.vector.memset(ones_row[:], 1.0)
        bv_bc = const.tile([128, DC], f32, name="bv_bc")

        # persistent activation tensors
        qT = [persist.tile([128, S], bf16, name=f"qT{m}") for m in range(4)]
        kT = [persist.tile([128, S], bf16, name=f"kT{m}") for m in range(4)]
        v_sb = [persist.tile([128, HC * VW], bf16, name=f"v{m}") for m in range(16)]

        # ---------------- phase 0+1: load, transpose, project ----------------
        with (
            tc.tile_pool(name="projin", bufs=1) as projin,
            tc.tile_pool(name="pj_psum", bufs=3, space="PSUM") as pj,
        ):
            # weights cast f32 -> bf16 during DMA
            wq_sb = [projin.tile([128, DC], bf16, name=f"wq{p}") for p in range(8)]
            wk_sb = [projin.tile([128, DC], bf16, name=f"wk{p}") for p in range(8)]
            wv_sb = [projin.tile([128, DC], bf16, name=f"wv{p}") for p in range(8)]
            for p in range(8):
                nc.gpsimd.dma_start(out=wq_sb[p][:], in_=wq_d[p * 128:(p + 1) * 128, :])
                nc.gpsimd.dma_start(out=wk_sb[p][:], in_=wk_d[p * 128:(p + 1) * 128, :])
                nc.gpsimd.dma_start(out=wv_sb[p][:], in_=wv_d[p * 128:(p + 1) * 128, :])

            # x cast-loaded to bf16 in natural layout, then transposed on the
            # TensorEngine into xT (8 transposes packed per PSUM bank via the
            # lazy-zero-region: start=True only on the first, stop on the last).
            ident = const.tile([128, 128], bf16, name="ident")
            make_identity(nc, ident[:])
            xT = [projin.tile([128, S], bf16, name=f"xT{p}") for p in range(8)]
            with tc.tile_pool(name="xnat", bufs=16) as xnp, \
                 tc.tile_pool(name="tr_psum", bufs=3, space="PSUM") as trp:
                xnat = []
                for m in range(16):
                    xn = xnp.tile([128, D], bf16, name=f"xn{m}", tag="xn")
                    nc.gpsimd.dma_start(out=xn[:], in_=x_d[m * 128:(m + 1) * 128, :])
                    xnat.append(xn)
                for p in range(8):
                    for half in range(2):
                        tp = trp.tile([128, 1024], bf16, name=f"tr{p}_{half}", tag="tr")
                        for mm in range(8):
                            m = half * 8 + mm
                            nc.tensor.matmul(
                                tp[:, mm * 128:(mm + 1) * 128],
                                xnat[m][:, p * 128:(p + 1) * 128],
                                ident[:],
                                is_transpose=True,
                                start=(mm == 0),
                                stop=(mm == 7),
                            )
                        nc.vector.tensor_copy(
                            xT[p][:, half * 1024:(half + 1) * 1024], tp[:]
                        )

            # bv broadcast to all 128 partitions via a rank-1 matmul
            bc_ps = pj.tile([128, DC], f32, name="bv_ps", tag="pj")
            nc.tensor.matmul(bc_ps[:], ones_row[:], bv_row[:], start=True, stop=True)
            nc.vector.tensor_copy(bv_bc[:], bc_ps[:])

            # qT[d', s] and kT[d', s]
            for wsb, bsb, dst in ((wq_sb, bq_sb, qT), (wk_sb, bk_sb, kT)):
                for m in range(4):
                    for n in range(4):
                        ps = pj.tile([128, 512], f32, name=f"pj_{m}_{n}", tag="pj")
                        for p in range(8):
                            nc.tensor.matmul(
                                ps[:],
                                wsb[p][:, m * 128:(m + 1) * 128],
                                xT[p][:, n * 512:(n + 1) * 512],
                                start=(p == 0),
                                stop=(p == 7),
                            )
                        nc.vector.tensor_scalar_add(
                            dst[m][:, n * 512:(n + 1) * 512], ps[:], bsb[:, m:m + 1]
                        )

            # v[s, d'] with the interleaved ones column per head
            for m in range(16):
                ones_ap = v_sb[m][:].rearrange("p (h c) -> p h c", c=VW)[:, :, HD:HD + 1]
                nc.vector.memset(ones_ap, 1.0)
                ps = pj.tile([128, 512], f32, name=f"pv_{m}", tag="pj")
                for p in range(8):
                    nc.tensor.matmul(
                        ps[:],
                        xT[p][:, m * 128:(m + 1) * 128],
                        wv_sb[p][:],
                        start=(p == 0),
                        stop=(p == 7),
                    )
                nc.vector.tensor_copy(
                    v_sb[m][:].rearrange("p (h c) -> p h c", c=VW)[:, :, 0:HD],
                    ps[:].rearrange("p (h c) -> p h c", c=HD),
                )

        # ---------------- phase 2: attention ----------------
        # Per (head-pair hp, 1024-wide query block jq):
        #   A) 16 key blocks: sT = kT_h^T qT_h (heads packed on PE row
        #      groups), e[ik][h01] = Exp(0.125*sT + mask) -> bf16 SBUF.
        #   B) per 128-query sub-block jj: one PSUM bank accumulates BOTH
        #      heads' [q,65] out_aug over all 16 key blocks (single
        #      accumulation chain per bank: start once, stop once).
        #   C) normalize by the ones-column and add bv into the staging
        #      tile; one big DMA per 512 query rows at the end of jq.
        with (
            tc.tile_pool(name="qk_psum", bufs=2, space="PSUM") as qkp,
            tc.tile_pool(name="pv_psum", bufs=3, space="PSUM") as pvp,
            tc.tile_pool(name="exp_sb", bufs=36) as ep,
            tc.tile_pool(name="stage", bufs=2) as stp,
            tc.tile_pool(name="recip", bufs=8) as rp,
        ):
            for jq in range(2):
                q0 = jq * 1024
                stage = [
                    stp.tile([128, 4 * DC], f32, name=f"st{jq}_{t}", tag=f"st{t}")
                    for t in range(2)
                ]
                for hp in range(4):
                    # --- A: scoresT + exp, both heads of the pair ---
                    e_tiles = []
                    for ik in range(16):
                        qk = [
                            qkp.tile([128, 1024], f32,
                                     name=f"qk{jq}_{hp}_{ik}_{i}", tag="qk")
                            for i in range(2)
                        ]
                        for half in range(2):
                            for h01 in range(2):
                                ro = h01 * 64
                                nc.tensor.matmul(
                                    qk[h01][:, half * 512:(half + 1) * 512],
                                    kT[hp][ro:ro + 64, ik * 128:(ik + 1) * 128],
                                    qT[hp][ro:ro + 64,
                                           q0 + half * 512:q0 + (half + 1) * 512],
                                    start=True,
                                    stop=True,
                                )
                        epair = []
                        for h01 in range(2):
                            e = ep.tile([128, 1024], bf16,
                                        name=f"e{jq}_{hp}_{ik}_{h01}", tag="e")
                            nc.scalar.activation(
                                e[:], qk[h01][:], AF.Exp,
                                bias=mask_sb[:, ik:ik + 1], scale=0.125,
                            )
                            epair.append(e)
                        e_tiles.append(epair)
                    # --- B+C: accumulate PV per 128-query sub-block ---
                    for jj in range(8):
                        t, j4 = divmod(jj, 4)
                        pv = pvp.tile([128, 2 * VW], f32,
                                      name=f"pv{jq}_{hp}_{jj}", tag="pv")
                        for h01 in range(2):
                            h = hp * 2 + h01
                            for ik in range(16):
                                nc.tensor.matmul(
                                    pv[:, h01 * VW:(h01 + 1) * VW],
                                    e_tiles[ik][h01][:, jj * 128:(jj + 1) * 128],
                                    v_sb[ik][:, h * VW:(h + 1) * VW],
                                    start=(h01 == 0 and ik == 0),
                                    stop=(h01 == 1 and ik == 15),
                                )
                        pv3 = pv[:].rearrange("p (g c) -> p g c", c=VW)
                        rc_t = rp.tile([128, 2, 1], f32,
                                       name=f"rc{jq}_{hp}_{jj}", tag="rc")
                        nc.vector.reciprocal(rc_t[:], pv3[:, :, HD:HD + 1])
                        dst = stage[t][:, j4 * DC + hp * 128:
                                       j4 * DC + (hp + 1) * 128].rearrange(
                            "p (g d) -> p g d", d=HD)
                        nc.vector.tensor_mul(
                            dst, pv3[:, :, 0:HD],
                            rc_t[:].broadcast_to([128, 2, HD]),
                        )
                        nc.vector.tensor_add(
                            dst, dst,
                            bv_bc[:, hp * 128:(hp + 1) * 128].rearrange(
                                "p (g d) -> p g d", d=HD),
                        )
                for t in range(2):
                    r0 = q0 + t * 512
                    nc.sync.dma_start(
                        out=out_d[r0:r0 + 512, :].rearrange("(j p) d -> p j d", p=128),
                        in_=stage[t][:].rearrange("p (j d) -> p j d", d=DC),
                    )


def _input_maps(input_tensor, attention_mask, Wq, bq, Wk, bk, Wv, bv):
    x = np.asarray(input_tensor, dtype=np.float32)
    m = np.asarray(attention_mask, dtype=np.float32)
    Wq = np.asarray(Wq, dtype=np.float32)
    Wk = np.asarray(Wk, dtype=np.float32)
    Wv = np.asarray(Wv, dtype=np.float32)
    bq = np.asarray(bq, dtype=np.float32)
    bk = np.asarray(bk, dtype=np.float32)
    bv = np.asarray(bv, dtype=np.float32)
    maps = []
    for c in range(NCORES):
        b, g = divmod(c, 2)
        cs = slice(g * DC, (g + 1) * DC)
        maps.append({
            "x": np.ascontiguousarray(x[b]),
            "mask": np.ascontiguousarray(m[b, 0, 0]),
            "wq": np.ascontiguousarray(Wq[:, cs]),
            "wk": np.ascontiguousarray(Wk[:, cs]),
            "wv": np.ascontiguousarray(Wv[:, cs]),
            "bq": np.ascontiguousarray(bq[cs]),
            "bk": np.ascontiguousarray(bk[cs]),
            "bv": np.ascontiguousarray(bv[cs]),
        })
    return maps


def get_nc() -> bass.Bass:
    if "nc" not in _cache:
        _cache["nc"] = _build()
    return _cache["nc"]


def _get_runner():
    """Build (once) a cached jitted SPMD executor for the Bass module.

    Mirrors bass2jax.run_bass_via_pjrt's multi-core branch, but keeps the
    jitted function and mesh alive so repeated kernel() calls skip
    re-tracing/compilation.
    """
    if "runner" in _cache:
        return _cache["runner"]
    import jax
    from jax.experimental.shard_map import shard_map
    from jax.sharding import Mesh, PartitionSpec

    from concourse import bass2jax, mybir as mb

    nc = get_nc()
    bass2jax.install_neuronx_cc_hook()

    partition_name = (
        nc.partition_id_tensor.name if nc.partition_id_tensor else None
    )
    in_names, out_names, out_avals = [], [], []
    for alloc in nc.m.functions[0].allocations:
        if not isinstance(alloc, mb.MemoryLocationSet):
            continue
        name = alloc.memorylocations[0].name
        if alloc.kind == "ExternalInput":
            if name != partition_name:
                in_names.append(name)
        elif alloc.kind == "ExternalOutput":
            out_names.append(name)
            out_avals.append(
                jax.core.ShapedArray(tuple(alloc.tensor_shape), mb.dt.np(alloc.dtype))
            )
    n_params = len(in_names)
    all_in_names = in_names + out_names
    if partition_name is not None:
        all_in_names = all_in_names + [partition_name]

    def _body(*args):
        operands = list(args)
        if partition_name is not None:
            operands.append(bass2jax.partition_id_tensor())
        outs = bass2jax._bass_exec_p.bind(
            *operands,
            out_avals=tuple(out_avals),
            in_names=tuple(all_in_names),
            out_names=tuple(out_names),
            lowering_input_output_aliases=(),
            sim_require_finite=True,
            sim_require_nnan=True,
            nc=nc,
        )
        return tuple(outs)

    devices = jax.devices()[:NCORES]
    mesh = Mesh(np.asarray(devices), ("core",))
    n_outs = len(out_names)
    sharded = jax.jit(
        shard_map(
            _body,
            mesh=mesh,
            in_specs=(PartitionSpec("core"),) * (n_params + n_outs),
            out_specs=(PartitionSpec("core"),) * n_outs,
            check_rep=False,
        ),
        donate_argnums=tuple(range(n_params, n_params + n_outs)),
        keep_unused=True,
    )
    zero_shapes = [
        (NCORES * a.shape[0], *a.shape[1:]) for a in out_avals
    ]
    zero_dtypes = [a.dtype for a in out_avals]

    def run(maps):
        concat_in = [
            np.concatenate([np.asarray(maps[c][nm]) for c in range(NCORES)], axis=0)
            for nm in in_names
        ]
        zeros = [np.zeros(s, d) for s, d in zip(zero_shapes, zero_dtypes)]
        out_arrs = sharded(*concat_in, *zeros)
        return [
            {
                nm: np.asarray(out_arrs[i]).reshape(NCORES, *out_avals[i].shape)[c]
                for i, nm in enumerate(out_names)
            }
            for c in range(NCORES)
        ]

    _cache["runner"] = run
    return run


def kernel(input_tensor, attention_mask, Wq, bq, Wk, bk, Wv, bv, _run_kwargs=None):
    maps = _input_maps(input_tensor, attention_mask, Wq, bq, Wk, bk, Wv, bv)
    if _run_kwargs:
        nc = get_nc()
        res = run_bass_kernel_spmd(nc, maps, list(range(NCORES)), **_run_kwargs)
        _cache["last_results"] = res
        results = res.results
    else:
        results = _get_runner()(maps)
    out = np.empty((B, S, D), dtype=np.float32)
    for c in range(NCORES):
        b, g = divmod(c, 2)
        out[b, :, g * DC:(g + 1) * DC] = results[c]["out"]
    return out
